# revision 7
# baseline (speedup 1.0000x reference)
"""Grouped-expert FFN (MoE) Trainium2 kernel.

Problem: E=64 experts, each x[1024,512] @ w1[512,2048] -> +b1 -> gelu(erf)
-> @ w2[2048,512] -> +b2, rows >= valid_load[e] zeroed.

Strategy:
 - Expert parallelism over 8 cores, 8 expert slots per core.
 - Host transposes x per expert (xT [D,C]) so the device contracts over D
   with zero on-chip transposes: GEMM1 computes hT = w1.T-tiles @ xT
   (stationary w1 k/m tile, moving xT), GEMM2 computes yT = w2-tiles @ hT.
   Both biases land on the partition axis -> free via ACT activation bias.
 - Rows >= valid_load are never computed: work is chunked in 512 columns of
   C; experts are snake-dealt to (core, slot) by descending chunk count so a
   single SPMD program (per-slot chunk counts = max over cores) is
   near-perfectly load balanced. Host assembles the full output with zeros.
 - All fp32 (PE streams fp32 at 1 elem/cell/cycle, same peak as bf16).
"""

import numpy as np

import concourse.bass as bass
import concourse.bacc as bacc
import concourse.tile as tile
from concourse import mybir
from concourse.bass_utils import run_bass_kernel_spmd

E, CAP, D, H = 64, 1024, 512, 2048
N_CORES = 8
SLOTS = E // N_CORES
CHUNK = 512                      # columns of C per work unit
MAX_CHUNKS = CAP // CHUNK        # 2
KTILES1 = D // 128               # 4  (contraction tiles of GEMM1)
MTILES1 = H // 128               # 16 (output partition tiles of GEMM1)
KTILES2 = H // 128               # 16 (contraction tiles of GEMM2)
MTILES2 = D // 128               # 4  (output partition tiles of GEMM2)

F32 = mybir.dt.float32
F32R = mybir.dt.float32r      # fast PE mode: 1 cyc/row (vs 4 for fp32) at N>=256
MM_DT = F32R                  # matmul operand dtype (bitcast view)

_PROGRAM_CACHE: dict[tuple, object] = {}
LAST_RESULT = None               # test harness introspection


def _build_program(slot_widths: tuple):
    """One SPMD program; slot s runs chunks of widths slot_widths[s]."""
    nc = bacc.Bacc(None, target_bir_lowering=False)

    xt = nc.dram_tensor("xt", [SLOTS, D, CAP], F32, kind="ExternalInput")
    w1g = nc.dram_tensor("w1g", [SLOTS, D, H], F32, kind="ExternalInput")
    w2g = nc.dram_tensor("w2g", [SLOTS, H, D], F32, kind="ExternalInput")
    b1g = nc.dram_tensor("b1g", [SLOTS, 128, MTILES1], F32, kind="ExternalInput")
    b2g = nc.dram_tensor("b2g", [SLOTS, 128, MTILES2], F32, kind="ExternalInput")
    yt = nc.dram_tensor("yt", [SLOTS, D, CAP], F32, kind="ExternalOutput")

    Gelu = mybir.ActivationFunctionType.Gelu
    Ident = mybir.ActivationFunctionType.Identity

    with tile.TileContext(nc) as tc:
        with (
            tc.tile_pool(name="w1p", bufs=2) as w1p,
            tc.tile_pool(name="w2p", bufs=2) as w2p,
            tc.tile_pool(name="bp", bufs=2) as bp,
            tc.tile_pool(name="xp", bufs=3) as xp,
            tc.tile_pool(name="hp", bufs=1) as hp,
            tc.tile_pool(name="yp", bufs=2) as yp,
            tc.tile_pool(name="ps_h", bufs=4, space="PSUM") as ps_h,
            tc.tile_pool(name="ps_y", bufs=4, space="PSUM") as ps_y,
        ):
            # interleave big/small slots (widths are sorted descending by
            # slot index) so DMA demand per compute window stays even
            emit_order = [0, 7, 1, 6, 2, 5, 3, 4][:SLOTS]
            for s in emit_order:
                widths = slot_widths[s]
                if not widths:
                    continue
                w1_t = w1p.tile([128, KTILES1, H], MM_DT, tag="w1")
                nc.sync.dma_start(
                    out=w1_t, in_=w1g[s].rearrange("(k p) h -> p k h", p=128).bitcast(MM_DT)
                )
                b1_t = bp.tile([128, MTILES1], F32, tag="b1")
                nc.sync.dma_start(out=b1_t, in_=b1g[s])
                b2_t = bp.tile([128, MTILES2], F32, tag="b2")
                nc.sync.dma_start(out=b2_t, in_=b2g[s])
                # w2 rides the second HWDGE ring (ACT) and is emitted after
                # the first x chunk: it is only needed once GEMM2 starts
                w2_t = w2p.tile([128, KTILES2, D], MM_DT, tag="w2")

                xt_s = xt[s].rearrange("(k p) c -> p k c", p=128)
                yt_s = yt[s].rearrange("(m p) c -> p m c", p=128)

                for j, W in enumerate(widths):
                    cs = slice(j * CHUNK, j * CHUNK + W)
                    x_t = xp.tile([128, KTILES1, CHUNK], MM_DT, tag="x")
                    nc.sync.dma_start(
                        out=x_t[:, :, :W], in_=xt_s[:, :, cs].bitcast(MM_DT)
                    )
                    if j == 0:
                        nc.scalar.dma_start(
                            out=w2_t,
                            in_=w2g[s].rearrange("(k p) d -> p k d", p=128).bitcast(MM_DT),
                        )

                    h_t = hp.tile([128, KTILES2, CHUNK], MM_DT, tag="h")
                    for m in range(MTILES1):
                        ps = ps_h.tile([128, CHUNK], F32, tag="psh")
                        for k in range(KTILES1):
                            nc.tensor.matmul(
                                ps[:, :W],
                                lhsT=w1_t[:, k, m * 128:(m + 1) * 128],
                                rhs=x_t[:, k, :W],
                                start=(k == 0),
                                stop=(k == KTILES1 - 1),
                            )
                        nc.scalar.activation(
                            h_t[:, m, :W], ps[:, :W], Gelu, bias=b1_t[:, m:m + 1]
                        )

                    y_t = yp.tile([128, MTILES2, CHUNK], F32, tag="y")
                    for dm in range(MTILES2):
                        ps2 = ps_y.tile([128, CHUNK], F32, tag="psy")
                        for k in range(KTILES2):
                            nc.tensor.matmul(
                                ps2[:, :W],
                                lhsT=w2_t[:, k, dm * 128:(dm + 1) * 128],
                                rhs=h_t[:, k, :W],
                                start=(k == 0),
                                stop=(k == KTILES2 - 1),
                            )
                        nc.scalar.activation(
                            y_t[:, dm, :W], ps2[:, :W], Ident, bias=b2_t[:, dm:dm + 1]
                        )
                    nc.gpsimd.dma_start(out=yt_s[:, :, cs], in_=y_t[:, :, :W])

    nc.compile()
    return nc


def kernel(packed_inputs, valid_load, w1, b1, w2, b2, _trace=False, **_):
    global LAST_RESULT
    packed_inputs = np.ascontiguousarray(np.asarray(packed_inputs, np.float32))
    w1 = np.asarray(w1, np.float32)
    b1 = np.asarray(b1, np.float32)
    w2 = np.asarray(w2, np.float32)
    b2 = np.asarray(b2, np.float32)
    v = np.asarray(valid_load).astype(np.int64)

    out = np.zeros((E, CAP, D), np.float32)
    if int(v.max()) <= 0:
        return out

    # snake-deal experts (sorted by descending valid rows) into core slots;
    # sorting by v keeps per-slot maxima tight so the compile-time tail
    # width (max over the 8 cores) wastes little work
    order = np.argsort(-v, kind="stable")
    assign = np.empty((N_CORES, SLOTS), np.int64)
    for s in range(SLOTS):
        blk = order[s * N_CORES:(s + 1) * N_CORES]
        assign[:, s] = blk if s % 2 == 0 else blk[::-1]

    slot_widths = []
    for s in range(SLOTS):
        mv = int(v[assign[:, s]].max())
        if mv <= 0:
            slot_widths.append(())
            continue
        nfull = (mv - 1) // CHUNK          # full 512 chunks before the tail
        tail = mv - nfull * CHUNK
        # fp32r needs moving dim >= 256 for the 1 cycle/row fast path
        tail = min(CHUNK, max(256, -(-tail // 32) * 32))
        slot_widths.append((CHUNK,) * nfull + (tail,))
    slot_widths = tuple(slot_widths)

    key = slot_widths
    if key not in _PROGRAM_CACHE:
        _PROGRAM_CACHE[key] = _build_program(slot_widths)
    nc = _PROGRAM_CACHE[key]

    in_maps = []
    for c in range(N_CORES):
        ids = assign[c]
        in_maps.append({
            "xt": np.ascontiguousarray(
                packed_inputs[ids].transpose(0, 2, 1)),
            "w1g": np.ascontiguousarray(w1[ids]),
            "w2g": np.ascontiguousarray(w2[ids]),
            "b1g": np.ascontiguousarray(
                b1[ids].reshape(SLOTS, MTILES1, 128).transpose(0, 2, 1)),
            "b2g": np.ascontiguousarray(
                b2[ids].reshape(SLOTS, MTILES2, 128).transpose(0, 2, 1)),
        })

    res = run_bass_kernel_spmd(nc, in_maps, list(range(N_CORES)), trace=_trace)
    LAST_RESULT = res

    for c in range(N_CORES):
        ytc = res.results[c]["yt"]
        for s in range(SLOTS):
            e = int(assign[c, s])
            ve = int(v[e])
            if ve > 0:
                out[e, :ve, :] = ytc[s, :, :ve].T
    return out
